# revision 7
# baseline (speedup 1.0000x reference)
"""Trainium2 Bass kernel for nn_BinarizedConv2d (2-bit-packed weight stream).

Math: activation[d, o] = sum_k weight_noise[d, o, k] * x[d, k]
      out[d, o]        = activation[d, o] > bias_noise[d, o]
with D=128 directions, O=256 out channels, K=2304 reduction length.
Sharding: D split across 8 NeuronCores (16 directions per core), no
collectives.

Weights and x are 0/1 bits, so adjacent k-pairs are packed host-side into
ONE fp8 byte p = w_even + 2*w_odd (exact float values {0,1,2,3}), halving
the HBM weight stream to 4.72 MB/core (the kernel is HBM-bound). On-chip,
the second operand stream is reconstructed with a single DVE op: the
fp8e4m3 encodings of {0,1,2,3} are {0x00,0x38,0x40,0x44}, so (enc & 0x40)
is the encoding of 2*w_odd exactly. The AND runs on a uint16-bitcast view
(2 bytes/elem, single-src SBUF->SBUF 16-bit => DVE 4x perf mode).

Per direction, two accumulating matmul streams give the exact popcount:
  sum_m xe[m]*p[m] + (0.5*xo[m] - xe[m]) * q[m] = sum_k w[k]*x[k]
(xe/xo are the even/odd x bits, host-split; c2 = 0.5*xo - xe is one DVE op;
every partial product is an integer, so fp32 PSUM accumulation is exact).

The threshold is folded into PSUM by one tiny fp16 matmul per quad:
stationary selneg[j, m] = -1 iff m//32 == j, moving operand the per-quad
row of kf = floor(bias) (integers ~576, exact in fp16), accumulated LAST so
PSUM arithmetic stays all-integer. For integer activations,
  act > bias  <=>  act - floor(bias) > 0.5,
so the epilogue is a single-src (psum is_gt 0.5) -> uint8, with no bias
tile, no 512KB bias replication DMA, and only the PE semaphore to wait on.

DMA discipline (learned from traces): each dma_start costs ~0.7us of issue
time on its engine and the runtime has only ~8 HWDGE completion semaphores
(more outstanding DMAs => sem-reuse stalls that starve the rings), so the
kernel uses 12 DMAs total: x/coeff header + 10 weight chunks (small first
chunk so the PE starts ~10.5us, 1-tile last chunk so the PE finishes right
behind the stream) split across the two HWDGE rings in consume order, and
one merged 4KB result store.
"""

import numpy as np
import ml_dtypes

D = 128          # directions (ES population)
O = 256          # out channels
K = 2304         # flattened reduction length
NT = 9           # packed k-tiles of 128 (K/2 = 1152 pairs)
P = 128          # partitions
NCORES = 8
DPC = D // NCORES  # directions per core
NQ = DPC // 4      # quads per core

FP8 = ml_dtypes.float8_e4m3

_nc_cache = {}

# weight chunk schedule: (quad, tile0, tile1) in consume order
CHUNKS = [
    (0, 0, 2), (0, 2, 5), (0, 5, 9),
    (1, 0, 4), (1, 4, 9),
    (2, 0, 4), (2, 4, 9),
    (3, 0, 4), (3, 4, 8), (3, 8, 9),
]
# ring assignment (0 = sync, 1 = scalar), balancing bytes; consume order is
# monotone within each ring
RING_OF = [0, 1, 0, 1, 0, 1, 0, 1, 0, 1]


def _emit(tc, res_ap, wT_ap, xT_ap, hdr_ap):
    """Emit the per-core program into TileContext tc."""
    import concourse.mybir as mybir

    nc = tc.nc
    fp8 = mybir.dt.float8e4
    u16 = mybir.dt.uint16
    f16 = mybir.dt.float16
    f32 = mybir.dt.float32
    u8 = mybir.dt.uint8
    XN = DPC * NT  # 144 coefficient columns per stream

    with (
        tc.tile_pool(name="w", bufs=1) as wp,
        tc.tile_pool(name="small", bufs=1) as sp,
        tc.tile_pool(name="act", bufs=1) as ap_pool,
        tc.tile_pool(name="ps", bufs=1, space="PSUM") as pp,
    ):
        # x even/odd bit streams first on the SP ring (everything depends on
        # them): xeo[:, :XN] = xe, xeo[:, XN:] = xo.
        xeo = sp.tile([P, 2 * XN], fp8)
        nc.sync.dma_start(out=xeo[:], in_=xT_ap)
        # header on the ACT ring: kf = floor(bias) [4, NQ*O] ++ selneg [4,128]
        hdr = sp.tile([4, NQ * O + P], f16)
        nc.scalar.dma_start(out=hdr[:], in_=hdr_ap)

        ring = [nc.sync, nc.scalar]
        p_tiles = [wp.tile([P, NT * 4 * O], fp8, tag=f"p{q}", name=f"p_t{q}")
                   for q in range(NQ)]
        q_tiles = [wp.tile([P, NT * 4 * O], fp8, tag=f"q{q}", name=f"q_t{q}")
                   for q in range(NQ)]
        for ci, (qi, t0, t1) in enumerate(CHUNKS):
            c0, c1 = t0 * 4 * O, t1 * 4 * O
            ring[RING_OF[ci]].dma_start(
                out=p_tiles[qi][:, c0:c1], in_=wT_ap[qi][:, c0:c1]
            )

        # Coefficient stream for the derived operand: c2 = 0.5*xo - xe.
        c2 = sp.tile([P, XN], fp8)
        nc.vector.scalar_tensor_tensor(
            out=c2[:],
            in0=xeo[:, XN:],
            scalar=0.5,
            in1=xeo[:, :XN],
            op0=mybir.AluOpType.mult,
            op1=mybir.AluOpType.subtract,
        )

        res_all = ap_pool.tile([P, NQ * O], u8)
        ps_all = pp.tile([P, 8 * 2 * O], f32)

        def derive(qi, t0, t1):
            c0, c1 = t0 * 4 * O, t1 * 4 * O
            nc.vector.tensor_scalar(
                out=q_tiles[qi][:, c0:c1].bitcast(u16),
                in0=p_tiles[qi][:, c0:c1].bitcast(u16),
                scalar1=0x4040, scalar2=None,
                op0=mybir.AluOpType.bitwise_and,
            )

        def mm_quad(q):
            win = slice(q * 2 * O, q * 2 * O + O)
            for s in range(2):
                src_t = p_tiles[q] if s == 0 else q_tiles[q]
                coef = xeo if s == 0 else c2
                for t in range(NT):
                    for j in range(4):
                        d = q * 4 + j
                        nc.tensor.matmul(
                            ps_all[32 * j : 32 * (j + 1), win],
                            coef[:, d * NT + t : d * NT + t + 1].broadcast_to((P, 32)),
                            src_t[:, (t * 4 + j) * O : (t * 4 + j + 1) * O],
                            start=(s == 0 and t == 0),
                            stop=False,
                            tile_position=(0, 32 * j),
                            skip_group_check=True,
                        )
            # fold -floor(bias) into the (all-integer) accumulation, last
            nc.tensor.matmul(
                ps_all[:, win],
                hdr[0:4, NQ * O : NQ * O + P],
                hdr[0:4, q * O : (q + 1) * O],
                start=False,
                stop=True,
                skip_group_check=True,
            )

        def compare(q):
            nc.vector.tensor_scalar(
                out=res_all[:, q * O : (q + 1) * O],
                in0=ps_all[:, q * 2 * O : q * 2 * O + O],
                scalar1=0.5, scalar2=None,
                op0=mybir.AluOpType.is_gt,
            )

        # Emission = program order per engine; the interleave below keeps the
        # DVE FIFO as [c2, d(q0)x3, d(q1)x2, cmp0, d(q2)x2, cmp1, d(q3)x3,
        # cmp2, cmp3] so compares run as their quad finishes instead of
        # piling up behind the last derive on the kernel tail.
        for (qi, t0, t1) in CHUNKS[0:3]:
            derive(qi, t0, t1)
        mm_quad(0)
        for (qi, t0, t1) in CHUNKS[3:5]:
            derive(qi, t0, t1)
        compare(0)
        mm_quad(1)
        for (qi, t0, t1) in CHUNKS[5:7]:
            derive(qi, t0, t1)
        compare(1)
        mm_quad(2)
        for (qi, t0, t1) in CHUNKS[7:10]:
            derive(qi, t0, t1)
        compare(2)
        mm_quad(3)
        compare(3)

        # Single result store: rows 0,32,64,96 hold directions j=0..3.
        nc.sync.dma_start(out=res_ap[:, :], in_=res_all[0:P:32, :])


def _build():
    """Build the per-core Bass program (same NEFF on all 8 cores)."""
    import concourse.bacc as bacc
    import concourse.mybir as mybir
    from concourse.tile import TileContext

    nc = bacc.Bacc("TRN2", debug=False, enable_asserts=False)

    fp8 = mybir.dt.float8e4
    f16 = mybir.dt.float16
    u8 = mybir.dt.uint8

    # wT[q, p, (t*4 + j)*O + o] = packed pair stream for direction d0+4q+j,
    # pair index m = t*128 + p, value w[2m] + 2*w[2m+1] in fp8.
    wT = nc.dram_tensor("wT", [NQ, P, NT * 4 * O], fp8, kind="ExternalInput")
    # xT[p, s*144 + d*9 + t] = x[d0+d, 2*(t*128+p) + s] for s in {0=even,1=odd}
    xT = nc.dram_tensor("xT", [P, 2 * DPC * NT], fp8, kind="ExternalInput")
    # hdr[j, q*O + o] = floor(bias_noise[d0+4q+j, o]); hdr[j, NQ*O + m] =
    # -1.0 if m//32 == j else 0 (the bias-fold selector)
    hdr = nc.dram_tensor("hdr", [4, NQ * O + P], f16, kind="ExternalInput")
    # res[j, q*O + o] = out[d0+4q+j, o]
    res = nc.dram_tensor("res", [4, NQ * O], u8, kind="ExternalOutput")

    with TileContext(nc) as tc:
        _emit(tc, res.ap(), wT.ap(), xT.ap(), hdr.ap())
    nc.compile()
    return nc


def prepare_inputs(weight_noise, bias_noise, x):
    """Host-side dtype cast + pair packing + layout transform + sharding.

    All transforms are data-independent (fixed index shuffles and the exact
    0/1 -> fp8 pack; floor() of the threshold is an exact fp16 rewrite of
    the compare constant); the reduction/compare math runs on device.
    """
    w = np.asarray(weight_noise)                      # [D, O, K] 0/1 floats
    wpair = w.reshape(D, O, K // 2, 2)
    pvals = (wpair[..., 0] + 2.0 * wpair[..., 1]).astype(FP8)   # [D, O, 1152]
    # [D, O, NT, P] -> [D, P, NT, O]
    pT = np.ascontiguousarray(pvals.reshape(D, O, NT, P).transpose(0, 3, 2, 1))

    xb = np.asarray(x)
    xe = xb[:, 0::2].astype(FP8).reshape(D, NT, P)    # [D, NT, P]
    xo = xb[:, 1::2].astype(FP8).reshape(D, NT, P)
    xeT = np.ascontiguousarray(xe.transpose(2, 0, 1))  # [P, D, NT]
    xoT = np.ascontiguousarray(xo.transpose(2, 0, 1))

    kf = np.floor(np.asarray(bias_noise).astype(np.float64)).astype(np.float16)
    selneg = np.zeros((4, P), dtype=np.float16)
    for j in range(4):
        selneg[j, 32 * j : 32 * (j + 1)] = -1.0

    in_maps = []
    for c in range(NCORES):
        sl = slice(c * DPC, (c + 1) * DPC)
        # [d, p, t, o] -> [q, j, p, t, o] -> [q, p, t, j, o]
        wc = (
            pT[sl]
            .reshape(NQ, 4, P, NT, O)
            .transpose(0, 2, 3, 1, 4)
            .reshape(NQ, P, NT * 4 * O)
        )
        xc = np.concatenate(
            [xeT[:, sl, :].reshape(P, DPC * NT), xoT[:, sl, :].reshape(P, DPC * NT)],
            axis=1,
        )
        kc = (
            kf[sl]
            .reshape(NQ, 4, O)
            .transpose(1, 0, 2)
            .reshape(4, NQ * O)
        )
        hc = np.concatenate([kc, selneg], axis=1)
        in_maps.append(
            {
                "wT": np.ascontiguousarray(wc),
                "xT": np.ascontiguousarray(xc),
                "hdr": np.ascontiguousarray(hc),
            }
        )
    return in_maps


def run(weight_noise, bias_noise, x, trace=False, **spmd_kwargs):
    """Run on the 8 NeuronCores; returns (bool [D, O] output, results)."""
    from concourse.bass_utils import run_bass_kernel_spmd

    in_maps = prepare_inputs(weight_noise, bias_noise, x)
    if "nc" in _nc_cache:
        nc = _nc_cache["nc"]
    else:
        nc = _nc_cache["nc"] = _build()
    r = run_bass_kernel_spmd(
        nc, in_maps, core_ids=list(range(NCORES)), trace=trace, **spmd_kwargs
    )
    out = np.concatenate(
        [
            r.results[c]["res"]
            .reshape(4, NQ, O)
            .transpose(1, 0, 2)
            .reshape(DPC, O)
            for c in range(NCORES)
        ],
        axis=0,
    )
    return out.astype(bool), r


def kernel(weight_noise, bias_noise, x):
    out, _ = run(weight_noise, bias_noise, x)
    return out


# revision 8
# speedup vs baseline: 1.0255x; 1.0255x over previous
"""Trainium2 Bass kernel for nn_BinarizedConv2d (2-bit-packed weight stream).

Math: activation[d, o] = sum_k weight_noise[d, o, k] * x[d, k]
      out[d, o]        = activation[d, o] > bias_noise[d, o]
with D=128 directions, O=256 out channels, K=2304 reduction length.
Sharding: D split across 8 NeuronCores (16 directions per core), no
collectives.

Weights and x are 0/1 bits, so adjacent k-pairs are packed host-side into
ONE fp8 byte p = w_even + 2*w_odd (exact float values {0,1,2,3}), halving
the HBM weight stream to 4.72 MB/core (the kernel is HBM-bound). On-chip,
the second operand stream is reconstructed with a single DVE op: the
fp8e4m3 encodings of {0,1,2,3} are {0x00,0x38,0x40,0x44}, so (enc & 0x40)
is the encoding of 2*w_odd exactly. The AND runs on a uint16-bitcast view
(2 bytes/elem, single-src SBUF->SBUF 16-bit => DVE 4x perf mode).

Per direction, two accumulating matmul streams give the exact popcount:
  sum_m xe[m]*p[m] + (0.5*xo[m] - xe[m]) * q[m] = sum_k w[k]*x[k]
(xe/xo are the even/odd x bits, host-split; c2 = 0.5*xo - xe is one DVE op;
every partial product is an integer, so fp32 PSUM accumulation is exact).

The threshold is folded into PSUM by one tiny fp16 matmul per quad:
stationary selneg[j, m] = -1 iff m//32 == j, moving operand the per-quad
row of kf = floor(bias) (integers ~576, exact in fp16), accumulated LAST so
PSUM arithmetic stays all-integer. For integer activations,
  act > bias  <=>  act - floor(bias) > 0.5,
so the epilogue is a single-src (psum is_gt 0.5) -> uint8, with no bias
tile, no 512KB bias replication DMA, and only the PE semaphore to wait on.

DMA discipline (learned from traces): each dma_start costs ~0.7us of issue
time on its engine and the runtime has only ~8 HWDGE completion semaphores
(more outstanding DMAs => sem-reuse stalls that starve the rings), so the
kernel uses 12 DMAs total: x/coeff header + 10 weight chunks (small first
chunk so the PE starts ~10.5us, 1-tile last chunk so the PE finishes right
behind the stream) split across the two HWDGE rings in consume order, and
one merged 4KB result store.
"""

import numpy as np
import ml_dtypes

D = 128          # directions (ES population)
O = 256          # out channels
K = 2304         # flattened reduction length
NT = 9           # packed k-tiles of 128 (K/2 = 1152 pairs)
P = 128          # partitions
NCORES = 8
DPC = D // NCORES  # directions per core
NQ = DPC // 4      # quads per core

FP8 = ml_dtypes.float8_e4m3

_nc_cache = {}

# weight chunk schedule: (quad, tile0, tile1) in consume order
CHUNKS = [
    (0, 0, 2), (0, 2, 5), (0, 5, 9),
    (1, 0, 4), (1, 4, 9),
    (2, 0, 4), (2, 4, 9),
    (3, 0, 4), (3, 4, 8), (3, 8, 9),
]
# Ring assignment (0 = sync, 1 = scalar). Traced behavior: the scalar-ring
# queue ramps ~4us later than the sync-ring queue, so everything the PE
# needs early goes on sync and the scalar ring carries mid/late chunks.
# With issue order = consume order, each of the 8 HWDGE completion
# semaphores is only reused 8 DMAs later, when its first user has long
# completed - no reuse stalls (v3's alternating assignment chained a
# late-completing early chunk into a 7us issue stall of a late chunk).
RING_OF = [0, 0, 0, 0, 1, 0, 1, 0, 1, 0]


def _emit(tc, res_ap, wT_ap, xT_ap, hdr_ap):
    """Emit the per-core program into TileContext tc."""
    import concourse.mybir as mybir

    nc = tc.nc
    fp8 = mybir.dt.float8e4
    u16 = mybir.dt.uint16
    f16 = mybir.dt.float16
    f32 = mybir.dt.float32
    u8 = mybir.dt.uint8
    XN = DPC * NT  # 144 coefficient columns per stream

    with (
        tc.tile_pool(name="w", bufs=1) as wp,
        tc.tile_pool(name="small", bufs=1) as sp,
        tc.tile_pool(name="act", bufs=1) as ap_pool,
        tc.tile_pool(name="ps", bufs=1, space="PSUM") as pp,
    ):
        # x even/odd bit streams first on the SP ring (everything depends on
        # them): xeo[:, :XN] = xe, xeo[:, XN:] = xo.
        xeo = sp.tile([P, 2 * XN], fp8)
        nc.sync.dma_start(out=xeo[:], in_=xT_ap)
        # header on the ACT ring: kf = floor(bias) [4, NQ*O] ++ selneg [4,128]
        hdr = sp.tile([4, NQ * O + P], f16)
        nc.scalar.dma_start(out=hdr[:], in_=hdr_ap)

        ring = [nc.sync, nc.scalar]
        p_tiles = [wp.tile([P, NT * 4 * O], fp8, tag=f"p{q}", name=f"p_t{q}")
                   for q in range(NQ)]
        q_tiles = [wp.tile([P, NT * 4 * O], fp8, tag=f"q{q}", name=f"q_t{q}")
                   for q in range(NQ)]
        for ci, (qi, t0, t1) in enumerate(CHUNKS):
            c0, c1 = t0 * 4 * O, t1 * 4 * O
            ring[RING_OF[ci]].dma_start(
                out=p_tiles[qi][:, c0:c1], in_=wT_ap[qi][:, c0:c1]
            )

        # Coefficient stream for the derived operand: c2 = 0.5*xo - xe.
        c2 = sp.tile([P, XN], fp8)
        nc.vector.scalar_tensor_tensor(
            out=c2[:],
            in0=xeo[:, XN:],
            scalar=0.5,
            in1=xeo[:, :XN],
            op0=mybir.AluOpType.mult,
            op1=mybir.AluOpType.subtract,
        )

        res_all = ap_pool.tile([P, NQ * O], u8)
        ps_all = pp.tile([P, 8 * 2 * O], f32)

        def derive(qi, t0, t1):
            c0, c1 = t0 * 4 * O, t1 * 4 * O
            nc.vector.tensor_scalar(
                out=q_tiles[qi][:, c0:c1].bitcast(u16),
                in0=p_tiles[qi][:, c0:c1].bitcast(u16),
                scalar1=0x4040, scalar2=None,
                op0=mybir.AluOpType.bitwise_and,
            )

        def mm_quad(q):
            win = slice(q * 2 * O, q * 2 * O + O)
            for s in range(2):
                src_t = p_tiles[q] if s == 0 else q_tiles[q]
                coef = xeo if s == 0 else c2
                for t in range(NT):
                    for j in range(4):
                        d = q * 4 + j
                        nc.tensor.matmul(
                            ps_all[32 * j : 32 * (j + 1), win],
                            coef[:, d * NT + t : d * NT + t + 1].broadcast_to((P, 32)),
                            src_t[:, (t * 4 + j) * O : (t * 4 + j + 1) * O],
                            start=(s == 0 and t == 0),
                            stop=False,
                            tile_position=(0, 32 * j),
                            skip_group_check=True,
                        )
            # fold -floor(bias) into the (all-integer) accumulation, last
            nc.tensor.matmul(
                ps_all[:, win],
                hdr[0:4, NQ * O : NQ * O + P],
                hdr[0:4, q * O : (q + 1) * O],
                start=False,
                stop=True,
                skip_group_check=True,
            )

        def compare(q):
            nc.vector.tensor_scalar(
                out=res_all[:, q * O : (q + 1) * O],
                in0=ps_all[:, q * 2 * O : q * 2 * O + O],
                scalar1=0.5, scalar2=None,
                op0=mybir.AluOpType.is_gt,
            )

        # Emission = program order per engine; the interleave below keeps the
        # DVE FIFO as [c2, d(q0)x3, d(q1)x2, cmp0, d(q2)x2, cmp1, d(q3)x3,
        # cmp2, cmp3] so compares run as their quad finishes instead of
        # piling up behind the last derive on the kernel tail.
        for (qi, t0, t1) in CHUNKS[0:3]:
            derive(qi, t0, t1)
        mm_quad(0)
        for (qi, t0, t1) in CHUNKS[3:5]:
            derive(qi, t0, t1)
        compare(0)
        mm_quad(1)
        for (qi, t0, t1) in CHUNKS[5:7]:
            derive(qi, t0, t1)
        compare(1)
        mm_quad(2)
        for (qi, t0, t1) in CHUNKS[7:10]:
            derive(qi, t0, t1)
        compare(2)
        mm_quad(3)
        compare(3)

        # Single result store: rows 0,32,64,96 hold directions j=0..3.
        nc.sync.dma_start(out=res_ap[:, :], in_=res_all[0:P:32, :])


def _build():
    """Build the per-core Bass program (same NEFF on all 8 cores)."""
    import concourse.bacc as bacc
    import concourse.mybir as mybir
    from concourse.tile import TileContext

    nc = bacc.Bacc("TRN2", debug=False, enable_asserts=False)

    fp8 = mybir.dt.float8e4
    f16 = mybir.dt.float16
    u8 = mybir.dt.uint8

    # wT[q, p, (t*4 + j)*O + o] = packed pair stream for direction d0+4q+j,
    # pair index m = t*128 + p, value w[2m] + 2*w[2m+1] in fp8.
    wT = nc.dram_tensor("wT", [NQ, P, NT * 4 * O], fp8, kind="ExternalInput")
    # xT[p, s*144 + d*9 + t] = x[d0+d, 2*(t*128+p) + s] for s in {0=even,1=odd}
    xT = nc.dram_tensor("xT", [P, 2 * DPC * NT], fp8, kind="ExternalInput")
    # hdr[j, q*O + o] = floor(bias_noise[d0+4q+j, o]); hdr[j, NQ*O + m] =
    # -1.0 if m//32 == j else 0 (the bias-fold selector)
    hdr = nc.dram_tensor("hdr", [4, NQ * O + P], f16, kind="ExternalInput")
    # res[j, q*O + o] = out[d0+4q+j, o]
    res = nc.dram_tensor("res", [4, NQ * O], u8, kind="ExternalOutput")

    with TileContext(nc) as tc:
        _emit(tc, res.ap(), wT.ap(), xT.ap(), hdr.ap())
    nc.compile()
    return nc


def prepare_inputs(weight_noise, bias_noise, x):
    """Host-side dtype cast + pair packing + layout transform + sharding.

    All transforms are data-independent (fixed index shuffles and the exact
    0/1 -> fp8 pack; floor() of the threshold is an exact fp16 rewrite of
    the compare constant); the reduction/compare math runs on device.
    """
    w = np.asarray(weight_noise)                      # [D, O, K] 0/1 floats
    wpair = w.reshape(D, O, K // 2, 2)
    pvals = (wpair[..., 0] + 2.0 * wpair[..., 1]).astype(FP8)   # [D, O, 1152]
    # [D, O, NT, P] -> [D, P, NT, O]
    pT = np.ascontiguousarray(pvals.reshape(D, O, NT, P).transpose(0, 3, 2, 1))

    xb = np.asarray(x)
    xe = xb[:, 0::2].astype(FP8).reshape(D, NT, P)    # [D, NT, P]
    xo = xb[:, 1::2].astype(FP8).reshape(D, NT, P)
    xeT = np.ascontiguousarray(xe.transpose(2, 0, 1))  # [P, D, NT]
    xoT = np.ascontiguousarray(xo.transpose(2, 0, 1))

    kf = np.floor(np.asarray(bias_noise).astype(np.float64)).astype(np.float16)
    selneg = np.zeros((4, P), dtype=np.float16)
    for j in range(4):
        selneg[j, 32 * j : 32 * (j + 1)] = -1.0

    in_maps = []
    for c in range(NCORES):
        sl = slice(c * DPC, (c + 1) * DPC)
        # [d, p, t, o] -> [q, j, p, t, o] -> [q, p, t, j, o]
        wc = (
            pT[sl]
            .reshape(NQ, 4, P, NT, O)
            .transpose(0, 2, 3, 1, 4)
            .reshape(NQ, P, NT * 4 * O)
        )
        xc = np.concatenate(
            [xeT[:, sl, :].reshape(P, DPC * NT), xoT[:, sl, :].reshape(P, DPC * NT)],
            axis=1,
        )
        kc = (
            kf[sl]
            .reshape(NQ, 4, O)
            .transpose(1, 0, 2)
            .reshape(4, NQ * O)
        )
        hc = np.concatenate([kc, selneg], axis=1)
        in_maps.append(
            {
                "wT": np.ascontiguousarray(wc),
                "xT": np.ascontiguousarray(xc),
                "hdr": np.ascontiguousarray(hc),
            }
        )
    return in_maps


def run(weight_noise, bias_noise, x, trace=False, **spmd_kwargs):
    """Run on the 8 NeuronCores; returns (bool [D, O] output, results)."""
    from concourse.bass_utils import run_bass_kernel_spmd

    in_maps = prepare_inputs(weight_noise, bias_noise, x)
    if "nc" in _nc_cache:
        nc = _nc_cache["nc"]
    else:
        nc = _nc_cache["nc"] = _build()
    r = run_bass_kernel_spmd(
        nc, in_maps, core_ids=list(range(NCORES)), trace=trace, **spmd_kwargs
    )
    out = np.concatenate(
        [
            r.results[c]["res"]
            .reshape(4, NQ, O)
            .transpose(1, 0, 2)
            .reshape(DPC, O)
            for c in range(NCORES)
        ],
        axis=0,
    )
    return out.astype(bool), r


def kernel(weight_noise, bias_noise, x):
    out, _ = run(weight_noise, bias_noise, x)
    return out
